# revision 4
# baseline (speedup 1.0000x reference)
"""Segment min/max/mean kernel for TRN2 (8 NeuronCores, SPMD).

Problem: for each of two label maps (50k / 30k labels) over 4M pixels x 16
channels, compute per-label min / max / mean(=sum/sizes) per channel plus
s = exp(-sizes)-0.5, returning ([L1,49], [L2,49]).

Strategy (data-parallel over pixels, hint-compliant):
  - Host "sharding" step lays each map's pixels out into a label-major,
    fixed-slot padded layout: each label gets S slots per core (S tiered by
    label population; pixels of a label are round-robined across the 8
    cores). Pad slots replicate the label's first pixel value, which is
    neutral for min/max; sums subtract the pad contribution on-device
    (pad_count * first_value).
  - Each core streams its [rows, C, S] slabs and computes per-label partial
    min / max / sum with DVE windowed reduces (regular access patterns
    only).
  - Cross-core combine via on-device ReduceScatter collectives (CCE
    min / max / add); each core finalizes its shard: mean = sum * (1/sizes),
    s = exp(-sizes) - 0.5, and assembles [shard, 49].
  - Host concatenates the 8 shards and un-permutes the tiered label order.
"""
import math
import os

import numpy as np

import concourse.bass as bass
import concourse.bacc as bacc
import concourse.mybir as mybir
import concourse.tile as tile
from concourse.bass_utils import run_bass_kernel_spmd

P = 128
C = 16
NCORES = 8
ROW_ALIGN = 1024  # tiles of 128 x 8 equal shards

LAST_EXEC_NS = None

dt = mybir.dt
Alu = mybir.AluOpType


def _maybe_install_trace_hook():
    """Best-effort NTFF profile hook registration (for HW timing)."""
    try:
        import sys
        import types

        if "antenv.axon_hooks" in sys.modules:
            return True
        import antenv
        from trn_agent_boot.trn_boot import _ntff_profile_via_ctypes

        mod = types.ModuleType("antenv.axon_hooks")
        mod._hook = None
        mod.set_axon_ntff_profile_hook = lambda h: setattr(mod, "_hook", h)
        mod.get_axon_ntff_profile_hook = lambda: mod._hook
        sys.modules["antenv.axon_hooks"] = mod
        antenv.axon_hooks = mod
        hook = _ntff_profile_via_ctypes("/opt/axon/libaxon_pjrt.so")
        mod.set_axon_ntff_profile_hook(hook)
        import concourse.bass_utils as bu

        bu.upload_artifacts = lambda tmpdir: "local://" + tmpdir
        return hook is not None
    except Exception:
        return False


def _round_up(x, m):
    return (x + m - 1) // m * m


def _plan_tiers(counts):
    """Assign each label a slot count S (per core) and a row position.

    Returns (tier_meta, row_of_label, R) where tier_meta is a list of
    (S, off_rows, rows, labels_array) and R is the padded total row count.
    """
    L = len(counts)
    need = np.maximum((counts + NCORES - 1) // NCORES, 1).astype(np.int64)
    need = np.maximum(need, 2)
    order = np.argsort(need, kind="stable")
    sneed = need[order]
    # tier boundaries at quantiles of slot need
    qs = (0.55, 0.80, 0.95, 1.0)
    tiers = []
    lo = 0
    for q in qs:
        hi = int(round(L * q))
        if hi <= lo:
            continue
        S = int(sneed[hi - 1])
        if tiers and S == tiers[-1][0]:
            # merge equal-S tiers
            tiers[-1] = (S, np.concatenate([tiers[-1][1], order[lo:hi]]))
        else:
            tiers.append((S, order[lo:hi]))
        lo = hi
    # row layout: each tier padded to 128 rows; total padded to ROW_ALIGN
    tier_meta = []
    row_of_label = np.empty(L, np.int64)
    R = 0
    for i, (S, labs) in enumerate(tiers):
        rows = _round_up(len(labs), P)
        if i == len(tiers) - 1:
            rows = _round_up(R + rows, ROW_ALIGN) - R
        row_of_label[labs] = R + np.arange(len(labs))
        tier_meta.append((S, R, rows, labs))
        R += rows
    assert R % ROW_ALIGN == 0
    return tier_meta, row_of_label, R


def _prep_map(x, labels, L):
    """Build per-core streams and metadata for one label map."""
    N = len(labels)
    counts = np.bincount(labels, minlength=L).astype(np.int64)
    tier_meta, row_of_label, R = _plan_tiers(counts)

    order = np.argsort(labels, kind="stable")
    slab = labels[order]
    starts = np.zeros(L + 1, np.int64)
    np.cumsum(counts, out=starts[1:])
    rank = np.arange(N, dtype=np.int64) - starts[slab]
    core = (rank % NCORES).astype(np.int64)
    slot = (rank // NCORES).astype(np.int64)
    rowp = row_of_label[slab]

    first_vals = np.zeros((L, C), np.float32)
    nz = counts > 0
    first_vals[nz] = x[order[starts[:L][nz]]]

    tier_id_of_label = np.zeros(L, np.int64)
    for t, (_, _, _, labs) in enumerate(tier_meta):
        tier_id_of_label[labs] = t
    tier_of_pixel = tier_id_of_label[slab]

    # Per-core fill value: the core's own first assigned pixel (rank g),
    # falling back to the label's global first pixel when the core holds
    # none. Slot 0 then always equals the pad fill, so the sum correction
    # (pad_count * slot0) is exact.
    core_fill = np.zeros((NCORES, L, C), np.float32)
    for g in range(NCORES):
        idx_g = starts[:L] + np.minimum(g, np.maximum(counts - 1, 0))
        core_fill[g][nz] = x[order[idx_g[nz]]]

    streams = []  # per tier: [8, rows, C*S] float32
    padcnts = []  # per tier: [8, 128, rows//128] float32 (tiled layout)
    for t, (S, off, rows, labs) in enumerate(tier_meta):
        st = np.zeros((NCORES, rows, C, S), np.float32)
        st[:, : len(labs)] = core_fill[:, labs, :, None]
        sel = tier_of_pixel == t
        st[core[sel], rowp[sel] - off, :, slot[sel]] = x[order[sel]]
        streams.append(st.reshape(NCORES, rows, C * S))
        # pad count per (core, row): S - count_on_core
        cnt_rows = np.zeros((rows,), np.int64)
        cnt_rows[: len(labs)] = counts[labs]
        g = np.arange(NCORES, dtype=np.int64)[:, None]
        cnt_core = (cnt_rows[None, :] + NCORES - 1 - g) // NCORES  # [8, rows]
        pc = (S - cnt_core).astype(np.float32)
        padcnts.append(pc.reshape(NCORES, rows // P, P).transpose(0, 2, 1).copy())

    return {
        "R": R,
        "tier_meta": tier_meta,
        "row_of_label": row_of_label,
        "counts": counts,
        "streams": streams,
        "padcnts": padcnts,
    }


_PROGRAM_CACHE = {}


def _build_program(key, plans):
    """Build (or fetch cached) the SPMD Bacc program for the given tier plans."""
    if key in _PROGRAM_CACHE:
        return _PROGRAM_CACHE[key]

    nc = bacc.Bacc("TRN2", num_devices=NCORES)
    params = {}
    for m, plan in enumerate(plans):
        R = plan["R"]
        for t, (S, off, rows, _labs) in enumerate(plan["tier_meta"]):
            params[f"xs{m}_{t}"] = nc.declare_dram_parameter(
                f"xs{m}_{t}", [rows, C * S], dt.float32, isOutput=False)
            params[f"pc{m}_{t}"] = nc.declare_dram_parameter(
                f"pc{m}_{t}", [P, rows // P], dt.float32, isOutput=False)
        params[f"sz{m}"] = nc.declare_dram_parameter(
            f"sz{m}", [P, R // NCORES // P], dt.float32, isOutput=False)
        params[f"out{m}"] = nc.declare_dram_parameter(
            f"out{m}", [R // NCORES, 49], dt.float32, isOutput=True)

    with tile.TileContext(nc) as tc:
        with (
            tc.tile_pool(name="ld", bufs=4) as ld_pool,
            tc.tile_pool(name="small", bufs=4) as sm_pool,
            tc.tile_pool(name="persist", bufs=1) as ps_pool,
            tc.tile_pool(name="dram", bufs=1, space="DRAM") as dram_pool,
        ):
            for m, plan in enumerate(plans):
                R = plan["R"]
                n_tiles = R // P
                fsh = R // NCORES  # shard rows
                f_tiles = fsh // P

                pmn_sb = ps_pool.tile([P, n_tiles * C], dt.float32,
                                      name=f"pmn_sb{m}")
                pmx_sb = ps_pool.tile([P, n_tiles * C], dt.float32,
                                      name=f"pmx_sb{m}")
                psm_sb = ps_pool.tile([P, n_tiles * C], dt.float32,
                                      name=f"psm_sb{m}")

                gi = 0
                for t, (S, off, rows, _labs) in enumerate(plan["tier_meta"]):
                    xs = params[f"xs{m}_{t}"]
                    pc_par = params[f"pc{m}_{t}"]
                    pc_sb = ps_pool.tile([P, rows // P], dt.float32,
                                         name=f"pc_sb{m}_{t}")
                    nc.sync.dma_start(pc_sb[:], pc_par[:])
                    for i in range(rows // P):
                        tl = ld_pool.tile([P, C, S], dt.float32, tag="tl")
                        nc.sync.dma_start(
                            tl[:].rearrange("p c s -> p (c s)"),
                            xs[i * P:(i + 1) * P, :],
                        )
                        o_mn = pmn_sb[:, gi * C:(gi + 1) * C]
                        o_mx = pmx_sb[:, gi * C:(gi + 1) * C]
                        o_sm = psm_sb[:, gi * C:(gi + 1) * C]
                        nc.vector.tensor_reduce(
                            o_mn, tl[:], axis=mybir.AxisListType.X, op=Alu.min)
                        nc.vector.tensor_reduce(
                            o_mx, tl[:], axis=mybir.AxisListType.X, op=Alu.max)
                        nc.vector.tensor_reduce(
                            o_sm, tl[:], axis=mybir.AxisListType.X, op=Alu.add)
                        corr = sm_pool.tile([P, C], dt.float32, tag="corr")
                        nc.vector.tensor_scalar(
                            corr[:], tl[:, :, 0], pc_sb[:, i:i + 1], None,
                            op0=Alu.mult)
                        nc.vector.tensor_tensor(
                            o_sm, o_sm, corr[:], op=Alu.subtract)
                        gi += 1

                # partials to DRAM (one DMA per stat)
                pmn_dr = dram_pool.tile([R, C], dt.float32, name=f"pmn_dr{m}")
                pmx_dr = dram_pool.tile([R, C], dt.float32, name=f"pmx_dr{m}")
                psm_dr = dram_pool.tile([R, C], dt.float32, name=f"psm_dr{m}")
                for dr, sb in ((pmn_dr, pmn_sb), (pmx_dr, pmx_sb),
                               (psm_dr, psm_sb)):
                    nc.sync.dma_start(
                        dr[:].rearrange("(t p) c -> p t c", p=P),
                        sb[:].rearrange("p (t c) -> p t c", c=C))

                smn = dram_pool.tile([fsh, C], dt.float32, name=f"smn{m}")
                smx = dram_pool.tile([fsh, C], dt.float32, name=f"smx{m}")
                ssm = dram_pool.tile([fsh, C], dt.float32, name=f"ssm{m}")
                rg = [list(range(NCORES))]
                nc.gpsimd.collective_compute(
                    "ReduceScatter", Alu.min, replica_groups=rg,
                    ins=[pmn_dr.opt()], outs=[smn.opt()])
                nc.gpsimd.collective_compute(
                    "ReduceScatter", Alu.max, replica_groups=rg,
                    ins=[pmx_dr.opt()], outs=[smx.opt()])
                nc.gpsimd.collective_compute(
                    "ReduceScatter", Alu.add, replica_groups=rg,
                    ins=[psm_dr.opt()], outs=[ssm.opt()])

                # finalize shard
                fmn = ps_pool.tile([P, f_tiles * C], dt.float32,
                                   name=f"fmn{m}")
                fmx = ps_pool.tile([P, f_tiles * C], dt.float32,
                                   name=f"fmx{m}")
                fsm = ps_pool.tile([P, f_tiles * C], dt.float32,
                                   name=f"fsm{m}")
                fsz = ps_pool.tile([P, f_tiles], dt.float32, name=f"fsz{m}")
                nc.sync.dma_start(
                    fmn[:].rearrange("p (t c) -> p t c", c=C),
                    smn[:].rearrange("(t p) c -> p t c", p=P))
                nc.sync.dma_start(
                    fmx[:].rearrange("p (t c) -> p t c", c=C),
                    smx[:].rearrange("(t p) c -> p t c", p=P))
                nc.sync.dma_start(
                    fsm[:].rearrange("p (t c) -> p t c", c=C),
                    ssm[:].rearrange("(t p) c -> p t c", p=P))
                nc.sync.dma_start(fsz[:], params[f"sz{m}"][:])
                oas = ps_pool.tile([P, f_tiles * 49], dt.float32,
                                   name=f"oas{m}")
                for j in range(f_tiles):
                    rs = sm_pool.tile([P, 1], dt.float32, tag="rs")
                    nc.vector.reciprocal(rs[:], fsz[:, j:j + 1])
                    ob = oas[:, j * 49:(j + 1) * 49]
                    nc.vector.tensor_copy(ob[:, 0:16],
                                          fmn[:, j * C:(j + 1) * C])
                    nc.vector.tensor_copy(ob[:, 16:32],
                                          fmx[:, j * C:(j + 1) * C])
                    nc.vector.tensor_scalar(
                        ob[:, 32:48], fsm[:, j * C:(j + 1) * C], rs[:], None,
                        op0=Alu.mult)
                    ex = sm_pool.tile([P, 1], dt.float32, tag="ex")
                    nc.scalar.activation(
                        ex[:], fsz[:, j:j + 1],
                        mybir.ActivationFunctionType.Exp, scale=-1.0)
                    nc.vector.tensor_scalar(
                        ob[:, 48:49], ex[:], -0.5, None, op0=Alu.add)
                nc.sync.dma_start(
                    params[f"out{m}"][:].rearrange("(t p) c -> p t c", p=P),
                    oas[:].rearrange("p (t c) -> p t c", c=49))

    nc.finalize()
    _PROGRAM_CACHE[key] = nc
    return nc


def kernel(input, cell_1_mask, cell_2_mask, cell_1_sizes, cell_2_sizes,
           cell_1_count, cell_2_count):
    x = np.ascontiguousarray(np.asarray(input, dtype=np.float32))
    L1 = int(cell_1_count)
    L2 = int(cell_2_count)
    m1 = np.asarray(cell_1_mask).astype(np.int64)
    m2 = np.asarray(cell_2_mask).astype(np.int64)
    s1 = np.asarray(cell_1_sizes).astype(np.int32)
    s2 = np.asarray(cell_2_sizes).astype(np.int32)

    plans = [_prep_map(x, m1, L1), _prep_map(x, m2, L2)]
    sizes = [s1, s2]

    key = tuple(
        (m, t, S, rows)
        for m, plan in enumerate(plans)
        for t, (S, off, rows, _l) in enumerate(plan["tier_meta"])
    )
    nc = _build_program(key, plans)

    in_maps = [{} for _ in range(NCORES)]
    for m, plan in enumerate(plans):
        R = plan["R"]
        fsh = R // NCORES
        # sizes in row space
        sz_rows = np.zeros(R, np.float32)
        L = L1 if m == 0 else L2
        sz_rows[plan["row_of_label"]] = sizes[m][:L].astype(np.float32)
        for g in range(NCORES):
            for t, (S, off, rows, _labs) in enumerate(plan["tier_meta"]):
                in_maps[g][f"xs{m}_{t}"] = plan["streams"][t][g]
                in_maps[g][f"pc{m}_{t}"] = plan["padcnts"][t][g]
            shard = sz_rows[g * fsh:(g + 1) * fsh]
            in_maps[g][f"sz{m}"] = (
                shard.reshape(fsh // P, P).T.copy())

    trace = os.environ.get("BASS_KERNEL_TRACE", "0") == "1"
    if trace:
        trace = _maybe_install_trace_hook()
    res = run_bass_kernel_spmd(nc, in_maps, list(range(NCORES)), trace=trace)
    global LAST_EXEC_NS
    LAST_EXEC_NS = res.exec_time_ns

    outs = []
    for m, plan in enumerate(plans):
        R = plan["R"]
        L = L1 if m == 0 else L2
        full = np.concatenate(
            [res.results[g][f"out{m}"] for g in range(NCORES)], axis=0)
        outs.append(np.ascontiguousarray(full[plan["row_of_label"][:L]]))
    return outs[0], outs[1]


# revision 11
# speedup vs baseline: 1.1355x; 1.1355x over previous
"""Segment min/max/mean kernel for TRN2 (8 NeuronCores, SPMD).

Problem: for each of two label maps (50k / 30k labels) over 4M pixels x 16
channels, compute per-label min / max / mean(=sum/sizes) per channel plus
s = exp(-sizes)-0.5, returning ([L1,49], [L2,49]).

Strategy (data-parallel over pixels, per the sharding hint):
  - Host "sharding" lays each map's pixels out in a label-major, fixed-slot
    padded layout: each label gets S slots per core (S tiered by label
    population; a label's pixels are round-robined across the 8 cores).
    Pad slots replicate the core's first pixel of that label, which is
    neutral for min/max; sums subtract the pad contribution on-device
    (pad_count * slot0_value).
  - Each core streams its slabs (partition-major, 8-tile chunks) and
    computes per-label partial min / max / sum with DVE windowed reduces.
  - Cross-core combine via on-device ReduceScatter collectives (CCE
    min / max / add); each core finalizes its shard: mean = sum * (1/sizes),
    s = exp(-sizes) - 0.5, and assembles [shard, 49].
  - Host concatenates the 8 shards and un-permutes the tiered label order.
"""
import os

import numpy as np

import concourse.bass as bass
import concourse.bacc as bacc
import concourse.mybir as mybir
import concourse.tile as tile
from concourse.bass_utils import run_bass_kernel_spmd

P = 128
C = 16
NCORES = 8
CHUNK = 8                     # label-tiles per DMA/reduce chunk
ROW_ALIGN = P * CHUNK         # 1024: tiles of 128, 8 shards, chunk of 8

LAST_EXEC_NS = None

dt = mybir.dt
Alu = mybir.AluOpType


def _maybe_install_trace_hook():
    """Best-effort NTFF profile hook registration (for HW timing)."""
    try:
        import sys
        import types

        if "antenv.axon_hooks" in sys.modules:
            return True
        import antenv
        from trn_agent_boot.trn_boot import _ntff_profile_via_ctypes

        mod = types.ModuleType("antenv.axon_hooks")
        mod._hook = None
        mod.set_axon_ntff_profile_hook = lambda h: setattr(mod, "_hook", h)
        mod.get_axon_ntff_profile_hook = lambda: mod._hook
        sys.modules["antenv.axon_hooks"] = mod
        antenv.axon_hooks = mod
        hook = _ntff_profile_via_ctypes("/opt/axon/libaxon_pjrt.so")
        mod.set_axon_ntff_profile_hook(hook)
        import concourse.bass_utils as bu

        bu.upload_artifacts = lambda tmpdir: "local://" + tmpdir
        return hook is not None
    except Exception:
        return False


def _round_up(x, m):
    return (x + m - 1) // m * m


def _plan_tiers(counts):
    """Assign each label a per-core slot count S and a row position.

    Returns (tier_meta, row_of_label, R): tier_meta is a list of
    (S, off_rows, rows, labels_array); R is the padded total row count.
    """
    L = len(counts)
    need = np.maximum((counts + NCORES - 1) // NCORES, 2).astype(np.int64)
    order = np.argsort(need, kind="stable")
    sneed = need[order]
    qs = (0.55, 0.80, 0.95, 1.0)
    tiers = []
    lo = 0
    for q in qs:
        hi = int(round(L * q))
        if hi <= lo:
            continue
        S = int(sneed[hi - 1])
        if tiers and S == tiers[-1][0]:
            tiers[-1] = (S, np.concatenate([tiers[-1][1], order[lo:hi]]))
        else:
            tiers.append((S, order[lo:hi]))
        lo = hi
    tier_meta = []
    row_of_label = np.empty(L, np.int64)
    R = 0
    for S, labs in tiers:
        rows = _round_up(len(labs), ROW_ALIGN)
        row_of_label[labs] = R + np.arange(len(labs))
        tier_meta.append((S, R, rows, labs))
        R += rows
    assert R % ROW_ALIGN == 0
    return tier_meta, row_of_label, R


def _prep_map(x, labels, L):
    """Build per-core streams and metadata for one label map."""
    N = len(labels)
    counts = np.bincount(labels, minlength=L).astype(np.int64)
    tier_meta, row_of_label, R = _plan_tiers(counts)
    n_tiles = R // P

    order = np.argsort(labels, kind="stable")
    slab = labels[order]
    starts = np.zeros(L + 1, np.int64)
    np.cumsum(counts, out=starts[1:])
    rank = np.arange(N, dtype=np.int64) - starts[slab]
    core = (rank % NCORES).astype(np.int64)
    slot = (rank // NCORES).astype(np.int64)
    rowp = row_of_label[slab]

    nz = counts > 0
    # Per-core fill value: the core's own first assigned pixel (rank g),
    # falling back to the label's global first pixel when the core holds
    # none. Slot 0 then always equals the pad fill, so the sum correction
    # (pad_count * slot0) is exact.
    core_fill = np.zeros((NCORES, L, C), np.float32)
    for g in range(NCORES):
        idx_g = starts[:L] + np.minimum(g, np.maximum(counts - 1, 0))
        core_fill[g][nz] = x[order[idx_g[nz]]]

    tier_id_of_label = np.zeros(L, np.int64)
    for t, (_, _, _, labs) in enumerate(tier_meta):
        tier_id_of_label[labs] = t
    tier_of_pixel = tier_id_of_label[slab]

    streams = []   # per tier: [8, P, (rows//P) * C * S] float32 part-major
    pc16 = np.zeros((NCORES, P, n_tiles, C), np.float32)  # padcnt x16
    fv16 = np.zeros((NCORES, P, n_tiles, C), np.float32)  # slot0 values
    for t, (S, off, rows, labs) in enumerate(tier_meta):
        st = np.zeros((NCORES, rows, C, S), np.float32)
        st[:, : len(labs)] = core_fill[:, labs, :, None]
        sel = tier_of_pixel == t
        st[core[sel], rowp[sel] - off, :, slot[sel]] = x[order[sel]]
        # partition-major: [8, rows//P, P, C*S] -> [8, P, rows//P * C*S]
        stp = (st.reshape(NCORES, rows // P, P, C * S)
               .transpose(0, 2, 1, 3)
               .reshape(NCORES, P, (rows // P) * C * S))
        streams.append(np.ascontiguousarray(stp))

        cnt_rows = np.zeros((rows,), np.int64)
        cnt_rows[: len(labs)] = counts[labs]
        g = np.arange(NCORES, dtype=np.int64)[:, None]
        cnt_core = (cnt_rows[None, :] + NCORES - 1 - g) // NCORES  # [8, rows]
        pc = (S - cnt_core).astype(np.float32)                    # [8, rows]
        ti0 = off // P
        ntt = rows // P
        pc16[:, :, ti0:ti0 + ntt, :] = (
            pc.reshape(NCORES, ntt, P)[:, :, :, None].transpose(0, 2, 1, 3))
        fv_rows = np.zeros((NCORES, rows, C), np.float32)
        fv_rows[:, : len(labs)] = core_fill[:, labs]
        fv16[:, :, ti0:ti0 + ntt, :] = (
            fv_rows.reshape(NCORES, ntt, P, C).transpose(0, 2, 1, 3))

    return {
        "R": R,
        "tier_meta": tier_meta,
        "row_of_label": row_of_label,
        "counts": counts,
        "streams": streams,
        "pc16": pc16.reshape(NCORES, P, n_tiles * C),
        "fv16": fv16.reshape(NCORES, P, n_tiles * C),
    }


_PROGRAM_CACHE = {}


def _build_program(key, plans):
    if key in _PROGRAM_CACHE:
        return _PROGRAM_CACHE[key]

    nc = bacc.Bacc("TRN2", num_devices=NCORES)
    params = {}
    for m, plan in enumerate(plans):
        R = plan["R"]
        n_tiles = R // P
        for t, (S, off, rows, _labs) in enumerate(plan["tier_meta"]):
            params[f"xs{m}_{t}"] = nc.declare_dram_parameter(
                f"xs{m}_{t}", [P, (rows // P) * C * S], dt.float32,
                isOutput=False)
        params[f"pc{m}"] = nc.declare_dram_parameter(
            f"pc{m}", [P, n_tiles * C], dt.float32, isOutput=False)
        params[f"fv{m}"] = nc.declare_dram_parameter(
            f"fv{m}", [P, n_tiles * C], dt.float32, isOutput=False)
        fsh = R // NCORES
        params[f"sz{m}"] = nc.declare_dram_parameter(
            f"sz{m}", [P, fsh // P], dt.float32, isOutput=False)
        params[f"sz16_{m}"] = nc.declare_dram_parameter(
            f"sz16_{m}", [P, (fsh // P) * C], dt.float32, isOutput=False)
        params[f"out{m}"] = nc.declare_dram_parameter(
            f"out{m}", [fsh, 49], dt.float32, isOutput=True)

    with tile.TileContext(nc) as tc:
        with (
            tc.tile_pool(name="ld", bufs=4) as ld_pool,
            tc.tile_pool(name="aux", bufs=2) as aux_pool,
            tc.tile_pool(name="dram", bufs=1, space="DRAM") as dram_pool,
        ):
            for m, plan in enumerate(plans):
                R = plan["R"]
                n_tiles = R // P
                fsh = R // NCORES
                f_tiles = fsh // P

                # per-map pool so partial buffers are freed before the
                # next map's work needs the SBUF space
                ps_ctx = tc.tile_pool(name=f"persist{m}", bufs=1)
                ps_pool = ps_ctx.__enter__()
                pmn_sb = ps_pool.tile([P, n_tiles * C], dt.float32,
                                      name=f"pmn_sb{m}")
                pmx_sb = ps_pool.tile([P, n_tiles * C], dt.float32,
                                      name=f"pmx_sb{m}")
                psm_sb = ps_pool.tile([P, n_tiles * C], dt.float32,
                                      name=f"psm_sb{m}")

                gi = 0
                for t, (S, off, rows, _labs) in enumerate(plan["tier_meta"]):
                    xs = params[f"xs{m}_{t}"]
                    csz = C * S
                    for j in range(rows // P // CHUNK):
                        tl = ld_pool.tile([P, CHUNK, C, S], dt.float32,
                                          tag="tl")
                        nc.sync.dma_start(
                            tl[:].rearrange("p k c s -> p (k c s)"),
                            xs[:, j * CHUNK * csz:(j + 1) * CHUNK * csz],
                        )
                        sl = slice(gi * C, (gi + CHUNK) * C)
                        o_mn = pmn_sb[:, sl].rearrange(
                            "p (k c) -> p k c", c=C)
                        o_mx = pmx_sb[:, sl].rearrange(
                            "p (k c) -> p k c", c=C)
                        o_sm = psm_sb[:, sl].rearrange(
                            "p (k c) -> p k c", c=C)
                        nc.vector.tensor_reduce(
                            o_mn, tl[:], axis=mybir.AxisListType.X,
                            op=Alu.min)
                        nc.vector.tensor_reduce(
                            o_mx, tl[:], axis=mybir.AxisListType.X,
                            op=Alu.max)
                        nc.vector.tensor_reduce(
                            o_sm, tl[:], axis=mybir.AxisListType.X,
                            op=Alu.add)
                        gi += CHUNK

                # chunked pad correction: psm -= pc16 * fv16
                CCH = 64 * C  # 64 tiles per chunk
                for j in range(0, n_tiles * C, CCH):
                    w = min(CCH, n_tiles * C - j)
                    pc_sb = aux_pool.tile([P, CCH], dt.float32, tag="pcs")
                    fv_sb = aux_pool.tile([P, CCH], dt.float32, tag="fvs")
                    nc.sync.dma_start(pc_sb[:, :w], params[f"pc{m}"][:, j:j + w])
                    nc.sync.dma_start(fv_sb[:, :w], params[f"fv{m}"][:, j:j + w])
                    nc.vector.tensor_tensor(pc_sb[:, :w], pc_sb[:, :w],
                                            fv_sb[:, :w], op=Alu.mult)
                    nc.vector.tensor_tensor(psm_sb[:, j:j + w],
                                            psm_sb[:, j:j + w],
                                            pc_sb[:, :w], op=Alu.subtract)

                # partials to DRAM (one DMA per stat)
                pmn_dr = dram_pool.tile([R, C], dt.float32, name=f"pmn_dr{m}")
                pmx_dr = dram_pool.tile([R, C], dt.float32, name=f"pmx_dr{m}")
                psm_dr = dram_pool.tile([R, C], dt.float32, name=f"psm_dr{m}")
                for dr, sb in ((pmn_dr, pmn_sb), (pmx_dr, pmx_sb),
                               (psm_dr, psm_sb)):
                    nc.sync.dma_start(
                        dr[:].rearrange("(t p) c -> p t c", p=P),
                        sb[:].rearrange("p (t c) -> p t c", c=C))

                smn = dram_pool.tile([fsh, C], dt.float32, name=f"smn{m}")
                smx = dram_pool.tile([fsh, C], dt.float32, name=f"smx{m}")
                ssm = dram_pool.tile([fsh, C], dt.float32, name=f"ssm{m}")
                rg = [list(range(NCORES))]
                nc.gpsimd.collective_compute(
                    "ReduceScatter", Alu.min, replica_groups=rg,
                    ins=[pmn_dr.opt()], outs=[smn.opt()])
                nc.gpsimd.collective_compute(
                    "ReduceScatter", Alu.max, replica_groups=rg,
                    ins=[pmx_dr.opt()], outs=[smx.opt()])
                nc.gpsimd.collective_compute(
                    "ReduceScatter", Alu.add, replica_groups=rg,
                    ins=[psm_dr.opt()], outs=[ssm.opt()])

                # ---- finalize shard (batched ops) ----
                fmn = ps_pool.tile([P, f_tiles * C], dt.float32,
                                   name=f"fmn{m}")
                fmx = ps_pool.tile([P, f_tiles * C], dt.float32,
                                   name=f"fmx{m}")
                fsm = ps_pool.tile([P, f_tiles * C], dt.float32,
                                   name=f"fsm{m}")
                fsz = ps_pool.tile([P, f_tiles], dt.float32, name=f"fsz{m}")
                fsz16 = ps_pool.tile([P, f_tiles * C], dt.float32,
                                     name=f"fsz16{m}")
                nc.sync.dma_start(
                    fmn[:].rearrange("p (t c) -> p t c", c=C),
                    smn[:].rearrange("(t p) c -> p t c", p=P))
                nc.sync.dma_start(
                    fmx[:].rearrange("p (t c) -> p t c", c=C),
                    smx[:].rearrange("(t p) c -> p t c", p=P))
                nc.sync.dma_start(
                    fsm[:].rearrange("p (t c) -> p t c", c=C),
                    ssm[:].rearrange("(t p) c -> p t c", p=P))
                nc.sync.dma_start(fsz[:], params[f"sz{m}"][:])
                nc.sync.dma_start(fsz16[:], params[f"sz16_{m}"][:])

                oas = ps_pool.tile([P, f_tiles * 49], dt.float32,
                                   name=f"oas{m}")
                oas3 = oas[:].rearrange("p (t c) -> p t c", c=49)
                rcp16 = aux_pool.tile([P, f_tiles * C], dt.float32,
                                      tag="rcp16")
                nc.vector.reciprocal(rcp16[:], fsz16[:])
                nc.vector.tensor_copy(
                    oas3[:, :, 0:16],
                    fmn[:].rearrange("p (t c) -> p t c", c=C))
                nc.vector.tensor_copy(
                    oas3[:, :, 16:32],
                    fmx[:].rearrange("p (t c) -> p t c", c=C))
                nc.vector.tensor_tensor(
                    oas3[:, :, 32:48],
                    fsm[:].rearrange("p (t c) -> p t c", c=C),
                    rcp16[:].rearrange("p (t c) -> p t c", c=C),
                    op=Alu.mult)
                ex = aux_pool.tile([P, f_tiles], dt.float32, tag="ex")
                nc.scalar.activation(
                    ex[:], fsz[:], mybir.ActivationFunctionType.Exp,
                    scale=-1.0)
                nc.vector.tensor_scalar(
                    oas3[:, :, 48:49],
                    ex[:].rearrange("p (t o) -> p t o", o=1),
                    -0.5, None, op0=Alu.add)
                nc.sync.dma_start(
                    params[f"out{m}"][:].rearrange("(t p) c -> p t c", p=P),
                    oas3)
                ps_ctx.__exit__(None, None, None)

    nc.finalize()
    _PROGRAM_CACHE[key] = nc
    return nc


def kernel(input, cell_1_mask, cell_2_mask, cell_1_sizes, cell_2_sizes,
           cell_1_count, cell_2_count):
    x = np.ascontiguousarray(np.asarray(input, dtype=np.float32))
    L1 = int(cell_1_count)
    L2 = int(cell_2_count)
    m1 = np.asarray(cell_1_mask).astype(np.int64)
    m2 = np.asarray(cell_2_mask).astype(np.int64)
    s1 = np.asarray(cell_1_sizes).astype(np.int32)
    s2 = np.asarray(cell_2_sizes).astype(np.int32)

    plans = [_prep_map(x, m1, L1), _prep_map(x, m2, L2)]
    sizes = [s1, s2]

    key = tuple(
        (m, t, S, rows)
        for m, plan in enumerate(plans)
        for t, (S, off, rows, _l) in enumerate(plan["tier_meta"])
    )
    nc = _build_program(key, plans)

    in_maps = [{} for _ in range(NCORES)]
    for m, plan in enumerate(plans):
        R = plan["R"]
        fsh = R // NCORES
        L = L1 if m == 0 else L2
        sz_rows = np.zeros(R, np.float32)
        sz_rows[plan["row_of_label"]] = sizes[m][:L].astype(np.float32)
        for g in range(NCORES):
            for t in range(len(plan["tier_meta"])):
                in_maps[g][f"xs{m}_{t}"] = plan["streams"][t][g]
            in_maps[g][f"pc{m}"] = plan["pc16"][g]
            in_maps[g][f"fv{m}"] = plan["fv16"][g]
            shard = sz_rows[g * fsh:(g + 1) * fsh]
            sz_t = shard.reshape(fsh // P, P).T.copy()      # [P, f_tiles]
            in_maps[g][f"sz{m}"] = sz_t
            in_maps[g][f"sz16_{m}"] = np.repeat(
                sz_t[:, :, None], C, axis=2).reshape(P, -1)
    trace = os.environ.get("BASS_KERNEL_TRACE", "0") == "1"
    if trace:
        trace = _maybe_install_trace_hook()
    res = run_bass_kernel_spmd(nc, in_maps, list(range(NCORES)), trace=trace)
    global LAST_EXEC_NS
    LAST_EXEC_NS = res.exec_time_ns

    outs = []
    for m, plan in enumerate(plans):
        L = L1 if m == 0 else L2
        full = np.concatenate(
            [res.results[g][f"out{m}"] for g in range(NCORES)], axis=0)
        outs.append(np.ascontiguousarray(full[plan["row_of_label"][:L]]))
    return outs[0], outs[1]


# revision 13
# speedup vs baseline: 1.2980x; 1.1432x over previous
"""Segment min/max/mean kernel for TRN2 (8 NeuronCores, SPMD).

Problem: for each of two label maps (50k / 30k labels) over 4M pixels x 16
channels, compute per-label min / max / mean(=sum/sizes) per channel plus
s = exp(-sizes)-0.5, returning ([L1,49], [L2,49]).

Strategy (data-parallel over pixels, per the sharding hint):
  - Host "sharding" lays each map's pixels out in a label-major, fixed-slot
    padded layout: each label gets S slots per core (S tiered by label
    population; a label's pixels are round-robined across the 8 cores).
    Pad slots replicate the core's first pixel of that label, which is
    neutral for min/max; sums subtract the pad contribution on-device
    (pad_count * slot0_value).
  - Each core streams its slabs (partition-major, 8-tile chunks) and
    computes per-label partial min / max / sum with DVE windowed reduces.
  - Cross-core combine via on-device ReduceScatter collectives (CCE
    min / max / add); each core finalizes its shard: mean = sum * (1/sizes),
    s = exp(-sizes) - 0.5, and assembles [shard, 49].
  - Host concatenates the 8 shards and un-permutes the tiered label order.
"""
import os

import numpy as np

import concourse.bass as bass
import concourse.bacc as bacc
import concourse.mybir as mybir
import concourse.tile as tile
from concourse.bass_utils import run_bass_kernel_spmd

P = 128
C = 16
NCORES = 8
CHUNK = 8                     # label-tiles per DMA/reduce chunk
ROW_ALIGN = P * CHUNK         # 1024: tiles of 128, 8 shards, chunk of 8

LAST_EXEC_NS = None

dt = mybir.dt
Alu = mybir.AluOpType


def _maybe_install_trace_hook():
    """Best-effort NTFF profile hook registration (for HW timing)."""
    try:
        import sys
        import types

        if "antenv.axon_hooks" in sys.modules:
            return True
        import antenv
        from trn_agent_boot.trn_boot import _ntff_profile_via_ctypes

        mod = types.ModuleType("antenv.axon_hooks")
        mod._hook = None
        mod.set_axon_ntff_profile_hook = lambda h: setattr(mod, "_hook", h)
        mod.get_axon_ntff_profile_hook = lambda: mod._hook
        sys.modules["antenv.axon_hooks"] = mod
        antenv.axon_hooks = mod
        hook = _ntff_profile_via_ctypes("/opt/axon/libaxon_pjrt.so")
        mod.set_axon_ntff_profile_hook(hook)
        import concourse.bass_utils as bu

        bu.upload_artifacts = lambda tmpdir: "local://" + tmpdir
        return hook is not None
    except Exception:
        return False


def _round_up(x, m):
    return (x + m - 1) // m * m


def _plan_tiers(counts):
    """Assign each label a per-core slot count S and a row position.

    Returns (tier_meta, row_of_label, R): tier_meta is a list of
    (S, off_rows, rows, labels_array); R is the padded total row count.
    """
    L = len(counts)
    need = np.maximum((counts + NCORES - 1) // NCORES, 2).astype(np.int64)
    order = np.argsort(need, kind="stable")
    sneed = need[order]
    qs = (0.55, 0.80, 0.95, 1.0)
    tiers = []
    lo = 0
    for q in qs:
        hi = int(round(L * q))
        if hi <= lo:
            continue
        S = int(sneed[hi - 1])
        if tiers and S == tiers[-1][0]:
            tiers[-1] = (S, np.concatenate([tiers[-1][1], order[lo:hi]]))
        else:
            tiers.append((S, order[lo:hi]))
        lo = hi
    tier_meta = []
    row_of_label = np.empty(L, np.int64)
    R = 0
    for S, labs in tiers:
        rows = _round_up(len(labs), ROW_ALIGN)
        row_of_label[labs] = R + np.arange(len(labs))
        tier_meta.append((S, R, rows, labs))
        R += rows
    assert R % ROW_ALIGN == 0
    return tier_meta, row_of_label, R


def _prep_map(x, labels, L):
    """Build per-core streams and metadata for one label map."""
    N = len(labels)
    counts = np.bincount(labels, minlength=L).astype(np.int64)
    tier_meta, row_of_label, R = _plan_tiers(counts)
    n_tiles = R // P

    order = np.argsort(labels, kind="stable")
    slab = labels[order]
    starts = np.zeros(L + 1, np.int64)
    np.cumsum(counts, out=starts[1:])
    rank = np.arange(N, dtype=np.int64) - starts[slab]
    core = (rank % NCORES).astype(np.int64)
    slot = (rank // NCORES).astype(np.int64)
    rowp = row_of_label[slab]

    nz = counts > 0
    # Per-core fill value: the core's own first assigned pixel (rank g),
    # falling back to the label's global first pixel when the core holds
    # none. Slot 0 then always equals the pad fill, so the sum correction
    # (pad_count * slot0) is exact.
    core_fill = np.zeros((NCORES, L, C), np.float32)
    for g in range(NCORES):
        idx_g = starts[:L] + np.minimum(g, np.maximum(counts - 1, 0))
        core_fill[g][nz] = x[order[idx_g[nz]]]

    tier_id_of_label = np.zeros(L, np.int64)
    for t, (_, _, _, labs) in enumerate(tier_meta):
        tier_id_of_label[labs] = t
    tier_of_pixel = tier_id_of_label[slab]

    streams = []   # per tier: [8, P, (rows//P) * C * S] float32 part-major
    pc16 = np.zeros((NCORES, P, n_tiles, C), np.float32)  # padcnt x16
    fv16 = np.zeros((NCORES, P, n_tiles, C), np.float32)  # slot0 values
    for t, (S, off, rows, labs) in enumerate(tier_meta):
        st = np.zeros((NCORES, rows, C, S), np.float32)
        st[:, : len(labs)] = core_fill[:, labs, :, None]
        sel = tier_of_pixel == t
        st[core[sel], rowp[sel] - off, :, slot[sel]] = x[order[sel]]
        # partition-major: [8, rows//P, P, C*S] -> [8, P, rows//P * C*S]
        stp = (st.reshape(NCORES, rows // P, P, C * S)
               .transpose(0, 2, 1, 3)
               .reshape(NCORES, P, (rows // P) * C * S))
        streams.append(np.ascontiguousarray(stp))

        cnt_rows = np.zeros((rows,), np.int64)
        cnt_rows[: len(labs)] = counts[labs]
        g = np.arange(NCORES, dtype=np.int64)[:, None]
        cnt_core = (cnt_rows[None, :] + NCORES - 1 - g) // NCORES  # [8, rows]
        pc = (S - cnt_core).astype(np.float32)                    # [8, rows]
        ti0 = off // P
        ntt = rows // P
        pc16[:, :, ti0:ti0 + ntt, :] = (
            pc.reshape(NCORES, ntt, P)[:, :, :, None].transpose(0, 2, 1, 3))
        fv_rows = np.zeros((NCORES, rows, C), np.float32)
        fv_rows[:, : len(labs)] = core_fill[:, labs]
        fv16[:, :, ti0:ti0 + ntt, :] = (
            fv_rows.reshape(NCORES, ntt, P, C).transpose(0, 2, 1, 3))

    return {
        "R": R,
        "tier_meta": tier_meta,
        "row_of_label": row_of_label,
        "counts": counts,
        "streams": streams,
        "pc16": pc16.reshape(NCORES, P, n_tiles * C),
        "fv16": fv16.reshape(NCORES, P, n_tiles * C),
    }


_PROGRAM_CACHE = {}


def _build_program(key, plans):
    if key in _PROGRAM_CACHE:
        return _PROGRAM_CACHE[key]

    nc = bacc.Bacc("TRN2", num_devices=NCORES)
    params = {}
    for m, plan in enumerate(plans):
        R = plan["R"]
        n_tiles = R // P
        for t, (S, off, rows, _labs) in enumerate(plan["tier_meta"]):
            params[f"xs{m}_{t}"] = nc.declare_dram_parameter(
                f"xs{m}_{t}", [P, (rows // P) * C * S], dt.float32,
                isOutput=False)
        params[f"pc{m}"] = nc.declare_dram_parameter(
            f"pc{m}", [P, n_tiles * C], dt.float32, isOutput=False)
        params[f"fv{m}"] = nc.declare_dram_parameter(
            f"fv{m}", [P, n_tiles * C], dt.float32, isOutput=False)
        fsh = R // NCORES
        params[f"sz{m}"] = nc.declare_dram_parameter(
            f"sz{m}", [P, fsh // P], dt.float32, isOutput=False)
        params[f"sz16_{m}"] = nc.declare_dram_parameter(
            f"sz16_{m}", [P, (fsh // P) * C], dt.float32, isOutput=False)
        params[f"out{m}"] = nc.declare_dram_parameter(
            f"out{m}", [P, (fsh // P) * 49], dt.float32, isOutput=True)

    with tile.TileContext(nc) as tc:
        with (
            tc.tile_pool(name="ld", bufs=4) as ld_pool,
            tc.tile_pool(name="aux", bufs=2) as aux_pool,
            tc.tile_pool(name="dram", bufs=1, space="DRAM") as dram_pool,
        ):
            for m, plan in enumerate(plans):
                R = plan["R"]
                n_tiles = R // P
                fsh = R // NCORES
                f_tiles = fsh // P

                # per-map pool so partial buffers are freed before the
                # next map's work needs the SBUF space
                ps_ctx = tc.tile_pool(name=f"persist{m}", bufs=1)
                ps_pool = ps_ctx.__enter__()
                pmn_sb = ps_pool.tile([P, n_tiles * C], dt.float32,
                                      name=f"pmn_sb{m}")
                pmx_sb = ps_pool.tile([P, n_tiles * C], dt.float32,
                                      name=f"pmx_sb{m}")
                psm_sb = ps_pool.tile([P, n_tiles * C], dt.float32,
                                      name=f"psm_sb{m}")

                gi = 0
                for t, (S, off, rows, _labs) in enumerate(plan["tier_meta"]):
                    xs = params[f"xs{m}_{t}"]
                    csz = C * S
                    for j in range(rows // P // CHUNK):
                        tl = ld_pool.tile([P, CHUNK, C, S], dt.float32,
                                          tag="tl")
                        nc.sync.dma_start(
                            tl[:].rearrange("p k c s -> p (k c s)"),
                            xs[:, j * CHUNK * csz:(j + 1) * CHUNK * csz],
                        )
                        sl = slice(gi * C, (gi + CHUNK) * C)
                        o_mn = pmn_sb[:, sl].rearrange(
                            "p (k c) -> p k c", c=C)
                        o_mx = pmx_sb[:, sl].rearrange(
                            "p (k c) -> p k c", c=C)
                        o_sm = psm_sb[:, sl].rearrange(
                            "p (k c) -> p k c", c=C)
                        nc.vector.tensor_reduce(
                            o_mn, tl[:], axis=mybir.AxisListType.X,
                            op=Alu.min)
                        nc.vector.tensor_reduce(
                            o_mx, tl[:], axis=mybir.AxisListType.X,
                            op=Alu.max)
                        nc.vector.tensor_reduce(
                            o_sm, tl[:], axis=mybir.AxisListType.X,
                            op=Alu.add)
                        gi += CHUNK

                # chunked pad correction: psm -= pc16 * fv16
                CCH = 64 * C  # 64 tiles per chunk
                for j in range(0, n_tiles * C, CCH):
                    w = min(CCH, n_tiles * C - j)
                    pc_sb = aux_pool.tile([P, CCH], dt.float32, tag="pcs")
                    fv_sb = aux_pool.tile([P, CCH], dt.float32, tag="fvs")
                    nc.sync.dma_start(pc_sb[:, :w], params[f"pc{m}"][:, j:j + w])
                    nc.sync.dma_start(fv_sb[:, :w], params[f"fv{m}"][:, j:j + w])
                    nc.vector.tensor_tensor(pc_sb[:, :w], pc_sb[:, :w],
                                            fv_sb[:, :w], op=Alu.mult)
                    nc.vector.tensor_tensor(psm_sb[:, j:j + w],
                                            psm_sb[:, j:j + w],
                                            pc_sb[:, :w], op=Alu.subtract)

                # partials to DRAM, partition-major (one straight DMA per
                # stat, 128 large descriptors). The ReduceScatter then shards
                # along partitions: core g ends up with partitions
                # [16g, 16g+16) of the reduced [128, n_tiles*C] array; the
                # host accounts for that label mapping.
                pmn_dr = dram_pool.tile([P, n_tiles * C], dt.float32,
                                        name=f"pmn_dr{m}")
                pmx_dr = dram_pool.tile([P, n_tiles * C], dt.float32,
                                        name=f"pmx_dr{m}")
                psm_dr = dram_pool.tile([P, n_tiles * C], dt.float32,
                                        name=f"psm_dr{m}")
                nc.sync.dma_start(pmn_dr[:], pmn_sb[:])
                nc.sync.dma_start(pmx_dr[:], pmx_sb[:])
                nc.sync.dma_start(psm_dr[:], psm_sb[:])

                smn = dram_pool.tile([P // NCORES, n_tiles * C], dt.float32,
                                     name=f"smn{m}")
                smx = dram_pool.tile([P // NCORES, n_tiles * C], dt.float32,
                                     name=f"smx{m}")
                ssm = dram_pool.tile([P // NCORES, n_tiles * C], dt.float32,
                                     name=f"ssm{m}")
                rg = [list(range(NCORES))]
                nc.gpsimd.collective_compute(
                    "ReduceScatter", Alu.min, replica_groups=rg,
                    ins=[pmn_dr.opt()], outs=[smn.opt()])
                nc.gpsimd.collective_compute(
                    "ReduceScatter", Alu.max, replica_groups=rg,
                    ins=[pmx_dr.opt()], outs=[smx.opt()])
                nc.gpsimd.collective_compute(
                    "ReduceScatter", Alu.add, replica_groups=rg,
                    ins=[psm_dr.opt()], outs=[ssm.opt()])

                # ---- finalize shard (batched ops) ----
                fmn = ps_pool.tile([P, f_tiles * C], dt.float32,
                                   name=f"fmn{m}")
                fmx = ps_pool.tile([P, f_tiles * C], dt.float32,
                                   name=f"fmx{m}")
                fsm = ps_pool.tile([P, f_tiles * C], dt.float32,
                                   name=f"fsm{m}")
                fsz = ps_pool.tile([P, f_tiles], dt.float32, name=f"fsz{m}")
                fsz16 = ps_pool.tile([P, f_tiles * C], dt.float32,
                                     name=f"fsz16{m}")
                for fdst, fsrc in ((fmn, smn), (fmx, smx), (fsm, ssm)):
                    nc.sync.dma_start(
                        fdst[:],
                        fsrc[:].rearrange(
                            "q (t8 tt c) -> (q t8) (tt c)", t8=8, c=C))
                nc.sync.dma_start(fsz[:], params[f"sz{m}"][:])
                nc.sync.dma_start(fsz16[:], params[f"sz16_{m}"][:])

                oas = ps_pool.tile([P, f_tiles * 49], dt.float32,
                                   name=f"oas{m}")
                oas3 = oas[:].rearrange("p (t c) -> p t c", c=49)
                rcp16 = aux_pool.tile([P, f_tiles * C], dt.float32,
                                      tag="rcp16")
                nc.vector.reciprocal(rcp16[:], fsz16[:])
                nc.vector.tensor_copy(
                    oas3[:, :, 0:16],
                    fmn[:].rearrange("p (t c) -> p t c", c=C))
                nc.vector.tensor_copy(
                    oas3[:, :, 16:32],
                    fmx[:].rearrange("p (t c) -> p t c", c=C))
                nc.vector.tensor_tensor(
                    oas3[:, :, 32:48],
                    fsm[:].rearrange("p (t c) -> p t c", c=C),
                    rcp16[:].rearrange("p (t c) -> p t c", c=C),
                    op=Alu.mult)
                ex = aux_pool.tile([P, f_tiles], dt.float32, tag="ex")
                nc.scalar.activation(
                    ex[:], fsz[:], mybir.ActivationFunctionType.Exp,
                    scale=-1.0)
                nc.vector.tensor_scalar(
                    oas3[:, :, 48:49],
                    ex[:].rearrange("p (t o) -> p t o", o=1),
                    -0.5, None, op0=Alu.add)
                nc.sync.dma_start(params[f"out{m}"][:], oas[:])
                ps_ctx.__exit__(None, None, None)

    nc.finalize()
    _PROGRAM_CACHE[key] = nc
    return nc


def kernel(input, cell_1_mask, cell_2_mask, cell_1_sizes, cell_2_sizes,
           cell_1_count, cell_2_count):
    x = np.ascontiguousarray(np.asarray(input, dtype=np.float32))
    L1 = int(cell_1_count)
    L2 = int(cell_2_count)
    m1 = np.asarray(cell_1_mask).astype(np.int64)
    m2 = np.asarray(cell_2_mask).astype(np.int64)
    s1 = np.asarray(cell_1_sizes).astype(np.int32)
    s2 = np.asarray(cell_2_sizes).astype(np.int32)

    plans = [_prep_map(x, m1, L1), _prep_map(x, m2, L2)]
    sizes = [s1, s2]

    key = tuple(
        (m, t, S, rows)
        for m, plan in enumerate(plans)
        for t, (S, off, rows, _l) in enumerate(plan["tier_meta"])
    )
    nc = _build_program(key, plans)

    in_maps = [{} for _ in range(NCORES)]
    shard_rows = []  # per map: [8, P, f_tiles] global row index grids
    for m, plan in enumerate(plans):
        R = plan["R"]
        n_tiles = R // P
        f_tiles = n_tiles // NCORES
        L = L1 if m == 0 else L2
        sz_rows = np.zeros(R, np.float32)
        sz_rows[plan["row_of_label"]] = sizes[m][:L].astype(np.float32)
        # shard layout after partition-sharded ReduceScatter: core g's
        # finalize partition part=q*8+t8, column tt holds global row
        # r = (t8*f_tiles + tt)*128 + 16*g + q
        part = np.arange(P)
        q = part // 8
        t8 = part % 8
        tt = np.arange(f_tiles)
        rgrid = np.empty((NCORES, P, f_tiles), np.int64)
        for g in range(NCORES):
            rgrid[g] = ((t8[:, None] * f_tiles + tt[None, :]) * P
                        + 16 * g + q[:, None])
        shard_rows.append(rgrid)
        for g in range(NCORES):
            for t in range(len(plan["tier_meta"])):
                in_maps[g][f"xs{m}_{t}"] = plan["streams"][t][g]
            in_maps[g][f"pc{m}"] = plan["pc16"][g]
            in_maps[g][f"fv{m}"] = plan["fv16"][g]
            sz_t = sz_rows[rgrid[g]].astype(np.float32)     # [P, f_tiles]
            in_maps[g][f"sz{m}"] = sz_t
            in_maps[g][f"sz16_{m}"] = np.repeat(
                sz_t[:, :, None], C, axis=2).reshape(P, -1)
    trace = os.environ.get("BASS_KERNEL_TRACE", "0") == "1"
    if trace:
        trace = _maybe_install_trace_hook()
    res = run_bass_kernel_spmd(nc, in_maps, list(range(NCORES)), trace=trace)
    global LAST_EXEC_NS
    LAST_EXEC_NS = res.exec_time_ns

    outs = []
    for m, plan in enumerate(plans):
        R = plan["R"]
        n_tiles = R // P
        f_tiles = n_tiles // NCORES
        L = L1 if m == 0 else L2
        full = np.empty((R, 49), np.float32)
        for g in range(NCORES):
            og = res.results[g][f"out{m}"].reshape(P, f_tiles, 49)
            full[shard_rows[m][g].reshape(-1)] = og.reshape(-1, 49)
        outs.append(np.ascontiguousarray(full[plan["row_of_label"][:L]]))
    return outs[0], outs[1]
